# revision 1
# baseline (speedup 1.0000x reference)
"""Trainium2 Bass kernel for nn_AttentionModule (dense transformer block).

Computation (per batch element b):
    q = X @ Wq.T ; k = K @ Wk.T ; v = X @ Wv.T        (X=query_input, K=key_input)
    a = softmax((k @ q.T) / sqrt(D), axis=-1)          -> (NK, NQ)
    out = a @ v + K                                    -> (NK, D)

Sharding: data-parallel over batch, one batch element per NeuronCore (B == 8).

Layout strategy: matmul contractions run on the partition axis, so the host
pre-transposes X, K and the weights to feature-major layouts (and rounds them
to bf16 — partial sums stay fp32 in PSUM, and the residual add of key_input is
done in fp32, keeping output error at the ~1e-4 level). The kernel computes
qT/kT projections (kT and v spilled to DRAM), scores in [n_q, n_k] layout, exp
on the scalar engine, the softmax denominator with a ones-vector matmul, and
the context matmul consumes exp(S)T directly as the stationary operand. The
normalization is folded into the output pass as a fused per-partition
multiply-add on the vector engine.
"""

import numpy as np
import ml_dtypes

import concourse.tile as tile
from concourse import bacc, mybir
from concourse.bass_utils import run_bass_kernel_spmd
from concourse.masks import make_identity

B, NQ, NK, D = 8, 2048, 2048, 1024
P = 128
EB = D // P          # 8 feature blocks
NB = NQ // P         # 16 query-row blocks
MC = 512             # scores chunk width (n_k columns per chunk)
NMC = NK // MC       # 4 chunks
SCALE = 1.0 / float(np.sqrt(np.float32(D)))

F32 = mybir.dt.float32
BF16 = mybir.dt.bfloat16

_CACHE = {}


def _build():
    nc = bacc.Bacc("TRN2", target_bir_lowering=False, debug=False, num_devices=B)

    xT = nc.dram_tensor("xT", [D, NQ], BF16, kind="ExternalInput").ap()
    ktT = nc.dram_tensor("ktT", [D, NK], BF16, kind="ExternalInput").ap()
    knat = nc.dram_tensor("knat", [NK, D], F32, kind="ExternalInput").ap()
    wqT = nc.dram_tensor("wqT", [D, D], BF16, kind="ExternalInput").ap()
    wkT = nc.dram_tensor("wkT", [D, D], BF16, kind="ExternalInput").ap()
    wvT = nc.dram_tensor("wvT", [D, D], BF16, kind="ExternalInput").ap()
    out = nc.dram_tensor("out", [NK, D], F32, kind="ExternalOutput").ap()

    with tile.TileContext(nc) as tc:
        with (
            tc.tile_pool(name="const", bufs=1) as constp,
            tc.tile_pool(name="qt", bufs=EB) as qtp,
            tc.tile_pool(name="dram", bufs=1, space="DRAM") as dramp,
            tc.tile_pool(name="psum", bufs=1, space="PSUM") as psp,
            tc.tile_pool(name="stage", bufs=12) as stagep,
            tc.tile_pool(name="ktc", bufs=18) as ktcp,
        ):
            ident = constp.tile([1, 1], F32, tag="ident", name="ident")
            make_identity(nc, ident)
            ones = constp.tile([P, 1], BF16, tag="ones", name="ones")
            nc.vector.memset(ones, 1.0)

            kT_sp = dramp.tile([D, NK], BF16, tag="kT_sp", name="kT_sp")
            v_sp = dramp.tile([NQ, D], BF16, tag="v_sp", name="v_sp")

            qT = [qtp.tile([P, NQ], BF16, tag="qt", name="qt") for _ in range(EB)]

            # ---------------- phase 1: projections ----------------
            with (
                tc.tile_pool(name="bigin", bufs=16) as bigp,
                tc.tile_pool(name="wpool", bufs=16) as wp,
            ):
                # -- kT[e, m] = sum_d wkT[d, e] * ktT[d, m]  (spilled to DRAM)
                # ktT/wkT are loaded in column halves so the first matmul
                # group only waits on half the bytes (shorter pipeline fill).
                # loads are emitted in first-consumed order: wk first half,
                # then ktT quarters in consumption order, wk second half last.
                # The first matmul group only waits on ~2MB.
                wk_h = [[None] * 2 for _ in range(EB)]
                kt_q = [[None] * 4 for _ in range(EB)]
                for db in range(EB):
                    t = wp.tile([P, D // 2], BF16, tag="wh", name="wh", bufs=16)
                    nc.sync.dma_start(
                        out=t, in_=wkT[db * P:(db + 1) * P, 0:512]
                    )
                    wk_h[db][0] = t
                for q in range(4):
                    for db in range(EB):
                        t = bigp.tile([P, NK // 4], BF16, tag="kth", name="kth", bufs=32)
                        nc.sync.dma_start(
                            out=t,
                            in_=ktT[db * P:(db + 1) * P, q * 512:(q + 1) * 512],
                        )
                        kt_q[db][q] = t
                for db in range(EB):
                    t = wp.tile([P, D // 2], BF16, tag="wh", name="wh", bufs=16)
                    nc.sync.dma_start(
                        out=t, in_=wkT[db * P:(db + 1) * P, 512:1024]
                    )
                    wk_h[db][1] = t
                gi = 0
                for h2 in range(2):
                    for mc4 in range(NK // 512):
                        for eb in range(h2 * 4, h2 * 4 + 4):
                            tg = "mm" if gi % 2 == 0 else "st"
                            gi += 1
                            ps = psp.tile([P, 512], F32, tag=tg, name="mm",
                                          bufs=3 if tg == "mm" else 4)
                            for db in range(EB):
                                nc.tensor.matmul(
                                    ps,
                                    wk_h[db][h2][:, (eb % 4) * P:(eb % 4 + 1) * P],
                                    kt_q[db][mc4],
                                    start=(db == 0),
                                    stop=(db == EB - 1),
                                )
                            st = stagep.tile([P, 512], BF16, tag="stage", name="stage")
                            nc.vector.tensor_copy(st, ps)
                            nc.scalar.dma_start(
                                out=kT_sp[eb * P:(eb + 1) * P, mc4 * 512:(mc4 + 1) * 512],
                                in_=st,
                            )

                # prefetch chunk-0 score operands while qT/v phases run
                ktc0 = []
                for eb in range(EB):
                    t = ktcp.tile([P, MC], BF16, tag="ktc", name="ktc")
                    nc.sync.dma_start(out=t, in_=kT_sp[eb * P:(eb + 1) * P, 0:MC])
                    ktc0.append(t)

                # -- qT[e, n] = sum_d wqT[d, e] * xT[d, n]  (SBUF resident)
                x_in = []
                for db in range(EB):
                    t = bigp.tile([P, NQ], BF16, tag="big", name="big", bufs=8)
                    nc.sync.dma_start(out=t, in_=xT[db * P:(db + 1) * P, :])
                    x_in.append(t)
                wq = []
                for db in range(EB):
                    t = wp.tile([P, D], BF16, tag="w", name="w", bufs=16)
                    nc.sync.dma_start(out=t, in_=wqT[db * P:(db + 1) * P, :])
                    wq.append(t)
                for eb in range(EB):
                    for nc4 in range(NQ // 512):
                        tg = "mm" if (eb * 4 + nc4) % 2 == 0 else "st"
                        ps = psp.tile([P, 512], F32, tag=tg, name="mm",
                                      bufs=3 if tg == "mm" else 4)
                        for db in range(EB):
                            nc.tensor.matmul(
                                ps,
                                wq[db][:, eb * P:(eb + 1) * P],
                                x_in[db][:, nc4 * 512:(nc4 + 1) * 512],
                                start=(db == 0),
                                stop=(db == EB - 1),
                            )
                        nc.vector.tensor_copy(
                            qT[eb][:, nc4 * 512:(nc4 + 1) * 512], ps
                        )

                # -- v[n, dv] = sum_d xT[d, n] * wvT[d, dv]  (spilled to DRAM)
                wv = []
                for db in range(EB):
                    t = wp.tile([P, D], BF16, tag="w", name="w", bufs=16)
                    nc.sync.dma_start(out=t, in_=wvT[db * P:(db + 1) * P, :])
                    wv.append(t)
                for nb in range(NB):
                    for dc in range(D // 512):
                        tg = "mm" if (nb * 2 + dc) % 2 == 0 else "st"
                        ps = psp.tile([P, 512], F32, tag=tg, name="mm",
                                      bufs=3 if tg == "mm" else 4)
                        for db in range(EB):
                            nc.tensor.matmul(
                                ps,
                                x_in[db][:, nb * P:(nb + 1) * P],
                                wv[db][:, dc * 512:(dc + 1) * 512],
                                start=(db == 0),
                                stop=(db == EB - 1),
                            )
                        st = stagep.tile([P, 512], BF16, tag="stage", name="stage")
                        nc.vector.tensor_copy(st, ps)
                        nc.scalar.dma_start(
                            out=v_sp[nb * P:(nb + 1) * P, dc * 512:(dc + 1) * 512],
                            in_=st,
                        )

            # ---------------- phase 2: attention ----------------
            with (
                tc.tile_pool(name="expst", bufs=18) as expp,
                tc.tile_pool(name="vst", bufs=20) as vstp,
                tc.tile_pool(name="knp", bufs=6) as knp,
                tc.tile_pool(name="outp", bufs=6) as outp,
                tc.tile_pool(name="small", bufs=4) as smallp,
            ):
                for mc in range(NMC):
                    m0 = mc * MC
                    if mc == 0:
                        ktc = ktc0
                    else:
                        ktc = []
                        for eb in range(EB):
                            t = ktcp.tile([P, MC], BF16, tag="ktc", name="ktc")
                            nc.sync.dma_start(
                                out=t, in_=kT_sp[eb * P:(eb + 1) * P, m0:m0 + MC]
                            )
                            ktc.append(t)

                    # scores + exp + column-sum accumulation
                    expst = []
                    cs_ps = psp.tile([1, MC], F32, tag="csrp", name="cs", bufs=1)
                    for nb in range(NB):
                        st_ps = psp.tile([P, MC], F32, tag="st", name="st", bufs=4)
                        for eb in range(EB):
                            nc.tensor.matmul(
                                st_ps,
                                qT[eb][:, nb * P:(nb + 1) * P],
                                ktc[eb],
                                start=(eb == 0),
                                stop=(eb == EB - 1),
                            )
                        et = expp.tile([P, MC], BF16, tag="expst", name="expst")
                        nc.scalar.activation(
                            out=et, in_=st_ps,
                            func=mybir.ActivationFunctionType.Exp, scale=SCALE,
                        )
                        expst.append(et)
                        # the column-sum matmul for block j is emitted two
                        # groups late so the exp -> cs semaphore never gates PE
                        if nb >= 2:
                            j = nb - 2
                            nc.tensor.matmul(
                                cs_ps, ones, expst[j],
                                start=(j == 0), stop=False,
                            )

                    for j in (NB - 2, NB - 1):
                        nc.tensor.matmul(
                            cs_ps, ones, expst[j],
                            start=False, stop=(j == NB - 1),
                        )
                    recip_row = smallp.tile([1, MC], F32, tag="rrow", name="rrow")
                    nc.vector.reciprocal(recip_row, cs_ps)
                    rp_ps = psp.tile([P, MC // P], F32, tag="csrp", name="rp", bufs=1)
                    for j in range(MC // P):
                        nc.tensor.transpose(
                            rp_ps[:, j:j + 1],
                            recip_row[:, j * P:(j + 1) * P],
                            ident,
                        )
                    recip_pp = smallp.tile([P, MC // P], F32, tag="rpp", name="rpp")
                    nc.vector.tensor_copy(recip_pp, rp_ps)

                    # context: C[m, dv] = sum_n expst[n, m] * v[n, dv]
                    vts = []
                    for nb in range(NB):
                        vt = vstp.tile([P, D], BF16, tag="vst", name="vst")
                        nc.sync.dma_start(
                            out=vt, in_=v_sp[nb * P:(nb + 1) * P, :],
                        )
                        vts.append(vt)
                    for msb in range(MC // P):
                        r0 = m0 + msb * P
                        kn = knp.tile([P, D], F32, tag="knat", name="knat")
                        nc.sync.dma_start(out=kn, in_=knat[r0:r0 + P, :])
                        ot = outp.tile([P, D], F32, tag="ostage", name="ostage")
                        for dc in range(D // 512):
                            c_ps = psp.tile([P, 512], F32, tag="mm", name="mm", bufs=3)
                            for nb in range(NB):
                                nc.tensor.matmul(
                                    c_ps,
                                    expst[nb][:, msb * P:(msb + 1) * P],
                                    vts[nb][:, dc * 512:(dc + 1) * 512],
                                    start=(nb == 0),
                                    stop=(nb == NB - 1),
                                )
                            nc.vector.scalar_tensor_tensor(
                                out=ot[:, dc * 512:(dc + 1) * 512],
                                in0=c_ps,
                                scalar=recip_pp[:, msb:msb + 1],
                                in1=kn[:, dc * 512:(dc + 1) * 512],
                                op0=mybir.AluOpType.mult,
                                op1=mybir.AluOpType.add,
                            )
                        nc.scalar.dma_start(out=out[r0:r0 + P, :], in_=ot)

    nc.compile()
    return nc


def _get_nc():
    if "nc" not in _CACHE:
        _CACHE["nc"] = _build()
    return _CACHE["nc"]


def kernel(query_input, key_input, Wq, Wk, Wv):
    nc = _get_nc()
    bf = ml_dtypes.bfloat16
    query_input = np.asarray(query_input, dtype=np.float32)
    key_input = np.asarray(key_input, dtype=np.float32)
    Wq = np.asarray(Wq, dtype=np.float32)
    Wk = np.asarray(Wk, dtype=np.float32)
    Wv = np.asarray(Wv, dtype=np.float32)
    in_maps = []
    for b in range(B):
        in_maps.append({
            "xT": np.ascontiguousarray(query_input[b].T).astype(bf),
            "ktT": np.ascontiguousarray(key_input[b].T).astype(bf),
            "knat": np.ascontiguousarray(key_input[b]),
            "wqT": np.ascontiguousarray(Wq.T).astype(bf),
            "wkT": np.ascontiguousarray(Wk.T).astype(bf),
            "wvT": np.ascontiguousarray(Wv.T).astype(bf),
        })
    res = run_bass_kernel_spmd(nc, in_maps, list(range(B))).results
    return np.stack([res[b]["out"] for b in range(B)], axis=0)

